# revision 1
# baseline (speedup 1.0000x reference)
"""ConvLSTM block (B=16, T=16, 32->64ch, 64x64, 3x3 SAME conv) on 8 TRN2 cores.

Strategy: data-parallel over batch (2 images/core). Per timestep the 3x3 conv
over concat([x_t, h]) is computed as 9 shifted matmuls accumulating in PSUM
(contraction K=96 channels on partitions; moving operand = zero-padded
(H+2)x(W+2) image plane read at the tap offset). Gate channels are permuted
host-side to [i, f, g, o] so chunkA=[i;f] and chunkB=[g;o] land in partition
halves that keep every elementwise gate op lane-aligned; the single
cross-half addition (c = f*c + i*g) is bridged with one SBUF->SBUF DMA.
"""

import os
from contextlib import ExitStack

import numpy as np

import concourse.mybir as mybir
import concourse.tile as tile
from concourse import bacc
from concourse.bass_utils import run_bass_kernel_spmd

F32 = mybir.dt.float32
AF = mybir.ActivationFunctionType
ALU = mybir.AluOpType

# Problem shapes (hardcoded per harness contract).
B, T, CIN, HID, H, W = 16, 16, 32, 64, 64, 64
NCORES = 8
BL = B // NCORES            # images per core
CH = CIN + HID              # conv input channels
PH, PW = H + 2, W + 2       # zero-padded plane
RG_ROWS = 8                 # output rows per PSUM tile (8*64 = 512 = one bank)
NRG = H // RG_ROWS
NTAP = 9

# Matmul input dtype: float32r streams 1 row/cycle (vs 4 for float32) at N>=256.
_MM_DT_NAME = os.environ.get("CONVLSTM_MM_DT", "f32r")
MM_DT = {"f32r": mybir.dt.float32r, "f32": F32}[_MM_DT_NAME]


def _build(mm_dt=MM_DT, steps=T):
    nc = bacc.Bacc("TRN2", target_bir_lowering=False, debug=False)
    x_d = nc.dram_tensor("xin", [BL, T, CIN, H, W], F32, kind="ExternalInput")
    w_d = nc.dram_tensor("win", [CH, NTAP * 2 * 128], F32, kind="ExternalInput")
    b_d = nc.dram_tensor("bin", [128, 2], F32, kind="ExternalInput")
    o_d = nc.dram_tensor("out", [BL, HID, H, W], F32, kind="ExternalOutput")

    with tile.TileContext(nc) as tc:
        with ExitStack() as ctx:
            const = ctx.enter_context(tc.tile_pool(name="const", bufs=1))
            psum = ctx.enter_context(tc.tile_pool(name="psum", bufs=4, space="PSUM"))
            gp = ctx.enter_context(tc.tile_pool(name="gates", bufs=3))

            wsb_f = const.tile([128, NTAP * 2 * 128], F32, tag="wsb_f")
            nc.sync.dma_start(out=wsb_f[0:CH, :], in_=w_d[:, :])
            wsb = const.tile([128, NTAP * 2 * 128], mm_dt, tag="wsb")
            nc.vector.tensor_copy(wsb[0:CH, :], wsb_f[0:CH, :])
            bsb = const.tile([128, 2], F32, tag="bsb")
            nc.sync.dma_start(out=bsb[:, :], in_=b_d[:, :])

            # Ping-pong padded input planes: x_t in partitions [0,32),
            # h_t in [32,96). Borders stay zero after the initial memset.
            bufs = [
                const.tile([128, BL, PH, PW], mm_dt, tag=f"pbuf{i}", name=f"pbuf{i}")
                for i in range(2)
            ]
            # Cell state lives in partitions [64,128) (lane-aligned with f/o).
            cst = const.tile([128, BL, H * W], F32, tag="cst")
            for pb in bufs:
                nc.gpsimd.memset(pb[:, :, :, :].bitcast(F32), 0.0)
            nc.vector.memset(cst[:, :, :], 0.0)

            xst = ctx.enter_context(tc.tile_pool(name="xst", bufs=2))
            for t in range(steps):
                cur, nxt = bufs[t % 2], bufs[(t + 1) % 2]
                for img in range(BL):
                    # Stage x_t in fp32, then GPSIMD converts to fp32r while
                    # scattering into the padded plane (DMA cannot round).
                    xs = xst.tile([CIN, H, W], F32, tag="xs")
                    nc.sync.dma_start(out=xs[:, :, :], in_=x_d[img, t, :, :, :])
                    nc.gpsimd.tensor_copy(
                        cur[0:CIN, img, 1 : H + 1, 1 : W + 1], xs[:, :, :]
                    )
                for img in range(BL):
                    for rg in range(NRG):
                        y0 = rg * RG_ROWS
                        ps = [
                            psum.tile([128, RG_ROWS, 64], F32, tag=f"ps{c}", name=f"ps{c}")
                            for c in range(2)
                        ]
                        for c in range(2):
                            for tap in range(NTAP):
                                ky, kx = divmod(tap, 3)
                                nc.tensor.matmul(
                                    out=ps[c][:, :, :],
                                    lhsT=wsb[
                                        0:CH, (tap * 2 + c) * 128 : (tap * 2 + c + 1) * 128
                                    ],
                                    rhs=cur[
                                        0:CH, img, y0 + ky : y0 + ky + RG_ROWS, kx : kx + 64
                                    ],
                                    start=(tap == 0),
                                    stop=(tap == NTAP - 1),
                                )

                        csl = cst[64:128, img, y0 * 64 : (y0 + RG_ROWS) * 64]

                        sig_if = gp.tile([128, RG_ROWS, 64], F32, tag="sig_if")
                        nc.scalar.activation(
                            out=sig_if[:, :, :], in_=ps[0][:, :, :],
                            func=AF.Sigmoid, bias=bsb[:, 0:1],
                        )
                        tg_so = gp.tile([128, RG_ROWS, 64], F32, tag="tg_so")
                        nc.scalar.activation(
                            out=tg_so[0:64], in_=ps[1][0:64],
                            func=AF.Tanh, bias=bsb[0:64, 1:2],
                        )
                        nc.scalar.activation(
                            out=tg_so[64:128], in_=ps[1][64:128],
                            func=AF.Sigmoid, bias=bsb[64:128, 1:2],
                        )

                        p1 = gp.tile([128, RG_ROWS, 64], F32, tag="p1")
                        nc.vector.tensor_mul(p1[0:64], sig_if[0:64], tg_so[0:64])
                        # Bridge i*g from partitions [0,64) to [64,128).
                        nc.sync.dma_start(out=p1[64:128], in_=p1[0:64])
                        tmp = gp.tile([128, RG_ROWS, 64], F32, tag="tmp")
                        nc.vector.tensor_mul(tmp[64:128], sig_if[64:128], csl)
                        nc.vector.tensor_add(csl, tmp[64:128], p1[64:128])
                        tct = gp.tile([128, RG_ROWS, 64], F32, tag="tct")
                        nc.scalar.activation(tct[64:128], csl, func=AF.Tanh)

                        if t < steps - 1:
                            # h = sigmoid(o) * tanh(c), rounded to fp32r on
                            # write, then shifted to partitions [32,96).
                            htr = gp.tile([128, RG_ROWS, 64], mm_dt, tag="htr")
                            nc.vector.tensor_mul(
                                htr[64:128], tg_so[64:128], tct[64:128]
                            )
                            nc.sync.dma_start(
                                out=nxt[CIN : CIN + HID, img, y0 + 1 : y0 + 9, 1 : W + 1],
                                in_=htr[64:128, :, :],
                            )
                        else:
                            ht = gp.tile([128, RG_ROWS, 64], F32, tag="ht")
                            nc.vector.tensor_mul(
                                ht[64:128], tg_so[64:128], tct[64:128]
                            )
                            ost = gp.tile([128, RG_ROWS, 64], F32, tag="ost")
                            nc.vector.scalar_tensor_tensor(
                                out=ost[64:128], in0=ht[64:128], scalar=0.01,
                                in1=ht[64:128], op0=ALU.mult, op1=ALU.max,
                            )
                            nc.sync.dma_start(
                                out=o_d[img, :, y0 : y0 + RG_ROWS, :],
                                in_=ost[64:128, :, :],
                            )
    nc.compile()
    return nc


def _prep_weights(Wf, bf):
    # Gate order [i, f, o, g] -> [i, f, g, o]: chunkA=[i;f], chunkB=[g;o].
    perm = np.concatenate(
        [np.arange(128), np.arange(192, 256), np.arange(128, 192)]
    )
    Wp = np.asarray(Wf, np.float32)[perm]        # [256, CH, 3, 3]
    bp = np.asarray(bf, np.float32)[perm]
    # wl[cin, ky, kx, chunk, m] = Wp[chunk*128 + m, cin, ky, kx]
    wl = Wp.reshape(2, 128, CH, 3, 3).transpose(2, 3, 4, 0, 1)
    wl = np.ascontiguousarray(wl.reshape(CH, NTAP * 2 * 128))
    b2 = np.ascontiguousarray(bp.reshape(2, 128).T)  # [128, 2]
    return wl, b2


_NC_CACHE = {}


def _get_nc():
    key = _MM_DT_NAME
    if key not in _NC_CACHE:
        _NC_CACHE[key] = _build()
    return _NC_CACHE[key]


def _in_maps(x, Wf, bf):
    x = np.ascontiguousarray(np.asarray(x, np.float32))
    wl, b2 = _prep_weights(Wf, bf)
    return [
        {
            "xin": np.ascontiguousarray(x[i * BL : (i + 1) * BL]),
            "win": wl,
            "bin": b2,
        }
        for i in range(NCORES)
    ]


def _run(x, W, b, trace=False, **spmd_kwargs):
    nc = _get_nc()
    res = run_bass_kernel_spmd(
        nc, _in_maps(x, W, b), core_ids=list(range(NCORES)), trace=trace,
        **spmd_kwargs,
    )
    out = np.concatenate([res.results[i]["out"] for i in range(NCORES)], axis=0)
    return np.ascontiguousarray(out, dtype=np.float32), res


def kernel(x, W, b):
    out, _ = _run(x, W, b)
    return out



# revision 9
# speedup vs baseline: 1.3494x; 1.3494x over previous
"""ConvLSTM block (B=16, T=16, 32->64ch, 64x64, 3x3 SAME conv) on 8 TRN2 cores.

Data-parallel over batch (2 images/core). The 3x3 conv over concat([x_t, h])
contracts 96 ch x 9 taps = 864 planes; instead of 9 passes of K=96, the
planes are im2col-packed into 7 matmul passes of K<=128. Each pass's moving
operand is a [128, 64, 64] bf16 tile whose partitions hold pre-shifted tap
windows: h-tap windows are materialized per timestep as DVE copies (4x bf16
mode) from a zero-padded h plane kept in both partition halves; x-tap
windows are pre-shifted host-side and DMA-streamed per timestep. Gate
channels are permuted host-side to [i, f, g, o] so chunkA=[i;f], chunkB=
[g;o] keep the elementwise gate ops lane-aligned; the single cross-half
addition (c = f*c + i*g) is bridged with one SBUF->SBUF DMA per row-group.
"""

from contextlib import ExitStack

import ml_dtypes
import numpy as np

import concourse.mybir as mybir
import concourse.tile as tile
from concourse import bacc
from concourse.bass_utils import run_bass_kernel_spmd

F32 = mybir.dt.float32
BF16 = mybir.dt.bfloat16
AF = mybir.ActivationFunctionType
ALU = mybir.AluOpType

# Problem shapes (hardcoded per harness contract).
B, T, CIN, HID, H, W = 16, 16, 32, 64, 64, 64
NCORES = 8
BL = B // NCORES            # images per core
CH = CIN + HID              # conv input channels
PH, PW = H + 2, W + 2       # zero-padded plane
RG_ROWS = 8                 # output rows per PSUM tile (8*64 = 512 = one bank)
NRG = H // RG_ROWS
NG = 7                      # matmul passes (packed contraction groups)

# h-tap window placement: A-half = partitions [0,64), B-half = [64,128).
# Groups 0-3 pair one A-tap with one B-tap (K=128); group 4 pairs the last
# A-tap with two x-taps; groups 5/6 are x-only (K=128 / K=96).
A_TAPS = [(0, 0), (0, 2), (1, 1), (2, 0), (2, 2)]   # -> G0..G4 @ p0-63
B_TAPS = [(0, 1), (1, 0), (1, 2), (2, 1)]           # -> G0..G3 @ p64-127
X_TAPS = [(0, 0), (0, 1),                            # -> G4 @ p64-95, p96-127
          (0, 2), (1, 0), (1, 1), (1, 2),            # -> G5 @ p0-127 (32 each)
          (2, 0), (2, 1), (2, 2)]                    # -> G6 @ p0-95


def _build(steps=T):
    nc = bacc.Bacc("TRN2", target_bir_lowering=False, debug=False)
    xp_d = nc.dram_tensor("xin", [BL, T, 9 * CIN, H, W], BF16, kind="ExternalInput")
    w_d = nc.dram_tensor("win", [128, NG * 2 * 128], BF16, kind="ExternalInput")
    b_d = nc.dram_tensor("bin", [128, 2], F32, kind="ExternalInput")
    o_d = nc.dram_tensor("out", [BL, HID, H, W], F32, kind="ExternalOutput")

    with tile.TileContext(nc) as tc:
        with ExitStack() as ctx:
            const = ctx.enter_context(tc.tile_pool(name="const", bufs=1))
            psum = ctx.enter_context(tc.tile_pool(name="psum", bufs=4, space="PSUM"))
            gp = ctx.enter_context(tc.tile_pool(name="gates", bufs=8))

            wsb = const.tile([128, NG * 2 * 128], BF16, tag="wsb")
            nc.sync.dma_start(out=wsb[:, :], in_=w_d[:, :])
            bsb = const.tile([128, 2], F32, tag="bsb")
            nc.sync.dma_start(out=bsb[:, :], in_=b_d[:, :])

            # Packed moving-operand tiles, one per (pass, image).
            gts = [
                [
                    const.tile([128, H, W], BF16, tag=f"gt{g}_{img}", name=f"gt{g}_{img}")
                    for img in range(BL)
                ]
                for g in range(NG)
            ]
            # Padded h plane, kept identically in both partition halves:
            # [64,128) is written directly by the h = o*tanh(c) mul; [0,64)
            # is bridged with one DMA per (t, img).
            hpad = [
                const.tile([128, PH, PW], BF16, tag=f"hpad{img}", name=f"hpad{img}")
                for img in range(BL)
            ]
            # Cell state lives in partitions [64,128) (lane-aligned with f/o).
            cst = const.tile([128, BL, H, W], F32, tag="cst")
            for img in range(BL):
                nc.gpsimd.memset(hpad[img][:, :, :], 0.0)
            nc.vector.memset(cst[:, :, :, :], 0.0)

            def stage_x(t, img):
                """Load the pre-shifted x-tap windows for timestep t."""
                nc.sync.dma_start(out=gts[4][img][64:128, :, :],
                                  in_=xp_d[img, t, 0:64, :, :])
                nc.sync.dma_start(out=gts[5][img][0:128, :, :],
                                  in_=xp_d[img, t, 64:192, :, :])
                nc.sync.dma_start(out=gts[6][img][0:96, :, :],
                                  in_=xp_d[img, t, 192:288, :, :])

            def stage(t, img):
                """Build the h-tap windows for timestep t from h_{t-1}."""
                # Bridge the new h plane to the A-half first so the A-window
                # copies can start as soon as possible.
                nc.sync.dma_start(out=hpad[img][0:64, :, :],
                                  in_=hpad[img][64:128, :, :])
                for g, (ky, kx) in enumerate(B_TAPS):
                    nc.vector.tensor_copy(
                        gts[g][img][64:128, :, :],
                        hpad[img][64:128, ky : ky + H, kx : kx + W],
                    )
                for g, (ky, kx) in enumerate(A_TAPS):
                    nc.vector.tensor_copy(
                        gts[g][img][0:64, :, :],
                        hpad[img][0:64, ky : ky + H, kx : kx + W],
                    )
                stage_x(t, img)

            for img in range(BL):
                stage_x(0, img)

            for t in range(steps):
                for img in range(BL):
                    # Pass 1 over row-groups: matmuls, PSUM-draining
                    # activations, the i*g product, its cross-half DMA
                    # bridge, and the in-place c *= f. Batching the bridges
                    # lets their ~2.5us DMA latencies overlap instead of
                    # stalling the in-order DVE queue once per row-group.
                    p1s, tg_sos = [], []
                    for rg in range(NRG):
                        y0 = rg * RG_ROWS
                        ps = [
                            psum.tile([128, RG_ROWS, 64], F32, tag=f"ps{c}", name=f"ps{c}")
                            for c in range(2)
                        ]
                        # At t=0, h == 0: skip the pure-h passes G0-G3 and
                        # read only G4's x-half, so no h windows are needed.
                        g_lo = 4 if t == 0 else 0
                        for g in range(g_lo, NG):
                            p0 = 64 if (t == 0 and g == 4) else 0
                            p1_ = 96 if g == NG - 1 else 128
                            for c in range(2):
                                nc.tensor.matmul(
                                    out=ps[c][:, :, :],
                                    lhsT=wsb[p0:p1_, (g * 2 + c) * 128 : (g * 2 + c + 1) * 128],
                                    rhs=gts[g][img][p0:p1_, y0 : y0 + RG_ROWS, :],
                                    start=(g == g_lo),
                                    stop=(g == NG - 1),
                                )

                        csl = cst[64:128, img, y0 : y0 + RG_ROWS, :]

                        sig_if = gp.tile([128, RG_ROWS, 64], BF16, tag="sig_if", bufs=4)
                        nc.scalar.activation(
                            out=sig_if[:, :, :], in_=ps[0][:, :, :],
                            func=AF.Sigmoid, bias=bsb[:, 0:1],
                        )
                        tg_so = gp.tile([128, RG_ROWS, 64], BF16, tag="tg_so")
                        nc.scalar.activation(
                            out=tg_so[0:64], in_=ps[1][0:64],
                            func=AF.Tanh, bias=bsb[0:64, 1:2],
                        )
                        nc.scalar.activation(
                            out=tg_so[64:128], in_=ps[1][64:128],
                            func=AF.Sigmoid, bias=bsb[64:128, 1:2],
                        )

                        p1 = gp.tile([128, RG_ROWS, 64], BF16, tag="p1")
                        nc.vector.tensor_mul(p1[0:64], sig_if[0:64], tg_so[0:64])
                        # Bridge i*g from partitions [0,64) to [64,128).
                        nc.sync.dma_start(out=p1[64:128], in_=p1[0:64])
                        # c *= f in place (no tmp tile needed).
                        nc.gpsimd.tensor_mul(csl, sig_if[64:128], csl)
                        p1s.append(p1)
                        tg_sos.append(tg_so)

                    # Pass 2: c += i*g, tanh, h (or the final leaky output).
                    for rg in range(NRG):
                        y0 = rg * RG_ROWS
                        csl = cst[64:128, img, y0 : y0 + RG_ROWS, :]
                        nc.vector.tensor_add(csl, csl, p1s[rg][64:128])
                        tct = gp.tile([128, RG_ROWS, 64], BF16, tag="tct", bufs=4)
                        nc.scalar.activation(tct[64:128], csl, func=AF.Tanh)

                        if t < steps - 1:
                            # h = sigmoid(o)*tanh(c), written straight into the
                            # padded plane's B-half (same partitions as o).
                            nc.vector.tensor_mul(
                                hpad[img][64:128, 1 + y0 : 1 + y0 + RG_ROWS, 1 : 1 + W],
                                tg_sos[rg][64:128], tct[64:128],
                            )
                        else:
                            ht = gp.tile([128, RG_ROWS, 64], F32, tag="ht", bufs=2)
                            nc.vector.tensor_mul(
                                ht[64:128], tg_sos[rg][64:128], tct[64:128]
                            )
                            ost = gp.tile([128, RG_ROWS, 64], F32, tag="ost", bufs=2)
                            nc.vector.scalar_tensor_tensor(
                                out=ost[64:128], in0=ht[64:128], scalar=0.01,
                                in1=ht[64:128], op0=ALU.mult, op1=ALU.max,
                            )
                            nc.sync.dma_start(
                                out=o_d[img, :, y0 : y0 + RG_ROWS, :],
                                in_=ost[64:128, :, :],
                            )
                    if t < steps - 1:
                        stage(t + 1, img)
    nc.compile()
    return nc


def _prep_weights(Wf, bf):
    # Gate order [i, f, o, g] -> [i, f, g, o]: chunkA=[i;f], chunkB=[g;o].
    perm = np.concatenate(
        [np.arange(128), np.arange(192, 256), np.arange(128, 192)]
    )
    Wp = np.asarray(Wf, np.float32)[perm]        # [256, CH, 3, 3]
    bp = np.asarray(bf, np.float32)[perm]
    # wl[p, g, c, m] = Wp[c*128+m, ch(p), ky(p), kx(p)] per the group layout.
    wl = np.zeros((128, NG, 2, 128), np.float32)
    for c in range(2):
        blk = Wp[c * 128 : (c + 1) * 128]        # [128, CH, 3, 3]
        for g, (ky, kx) in enumerate(A_TAPS):
            wl[0:64, g, c, :] = blk[:, CIN:CH, ky, kx].T
        for g, (ky, kx) in enumerate(B_TAPS):
            wl[64:128, g, c, :] = blk[:, CIN:CH, ky, kx].T
        for j, (ky, kx) in enumerate(X_TAPS):
            g = 4 + (j + 2) // 4                 # 0,1 -> G4; 2-5 -> G5; 6-8 -> G6
            p0 = (64 + 32 * j) if j < 2 else 32 * ((j - 2) % 4)
            wl[p0 : p0 + CIN, g, c, :] = blk[:, 0:CIN, ky, kx].T
    wl = np.ascontiguousarray(
        wl.reshape(128, NG * 2 * 128).astype(ml_dtypes.bfloat16)
    )
    b2 = np.ascontiguousarray(bp.reshape(2, 128).T)  # [128, 2]
    return wl, b2


def _prep_x(x):
    """Pre-shifted x-tap windows: [B, T, 9*CIN, H, W] bf16, X_TAPS order."""
    x = np.asarray(x, np.float32)
    xpad = np.zeros((B, T, CIN, PH, PW), np.float32)
    xpad[:, :, :, 1 : 1 + H, 1 : 1 + W] = x
    wins = [xpad[:, :, :, ky : ky + H, kx : kx + W] for (ky, kx) in X_TAPS]
    xp = np.concatenate(wins, axis=2)
    return np.ascontiguousarray(xp.astype(ml_dtypes.bfloat16))


_NC_CACHE = {}


def _get_nc():
    if "nc" not in _NC_CACHE:
        _NC_CACHE["nc"] = _build()
    return _NC_CACHE["nc"]


def _in_maps(x, Wf, bf):
    xp = _prep_x(x)
    wl, b2 = _prep_weights(Wf, bf)
    return [
        {
            "xin": np.ascontiguousarray(xp[i * BL : (i + 1) * BL]),
            "win": wl,
            "bin": b2,
        }
        for i in range(NCORES)
    ]


def _run(x, W, b, trace=False, **spmd_kwargs):
    nc = _get_nc()
    res = run_bass_kernel_spmd(
        nc, _in_maps(x, W, b), core_ids=list(range(NCORES)), trace=trace,
        **spmd_kwargs,
    )
    out = np.concatenate([res.results[i]["out"] for i in range(NCORES)], axis=0)
    return np.ascontiguousarray(out, dtype=np.float32), res


def kernel(x, W, b):
    out, _ = _run(x, W, b)
    return out


# revision 12
# speedup vs baseline: 1.3501x; 1.0005x over previous
"""ConvLSTM block (B=16, T=16, 32->64ch, 64x64, 3x3 SAME conv) on 8 TRN2 cores.

Data-parallel over batch (2 images/core). The 3x3 conv over concat([x_t, h])
contracts 96 ch x 9 taps = 864 planes; instead of 9 passes of K=96, the
planes are im2col-packed into 7 matmul passes of K<=128. Each pass's moving
operand is a [128, 64, 64] bf16 tile whose partitions hold pre-shifted tap
windows: h-tap windows are materialized per timestep as DVE copies (4x bf16
mode) from a zero-padded h plane kept in both partition halves; x-tap
windows are pre-shifted host-side and DMA-streamed per timestep. Gate
channels are permuted host-side to [i, f, g, o] so chunkA=[i;f], chunkB=
[g;o] keep the elementwise gate ops lane-aligned; the single cross-half
addition (c = f*c + i*g) is bridged with one SBUF->SBUF DMA per row-group.
"""

from contextlib import ExitStack

import ml_dtypes
import numpy as np

import concourse.mybir as mybir
import concourse.tile as tile
from concourse import bacc
from concourse.bass_utils import run_bass_kernel_spmd

F32 = mybir.dt.float32
BF16 = mybir.dt.bfloat16
AF = mybir.ActivationFunctionType
ALU = mybir.AluOpType

# Problem shapes (hardcoded per harness contract).
B, T, CIN, HID, H, W = 16, 16, 32, 64, 64, 64
NCORES = 8
BL = B // NCORES            # images per core
CH = CIN + HID              # conv input channels
PH, PW = H + 2, W + 2       # zero-padded plane
RG_ROWS = 8                 # output rows per PSUM tile (8*64 = 512 = one bank)
NRG = H // RG_ROWS
NG = 7                      # matmul passes (packed contraction groups)

# h-tap window placement: A-half = partitions [0,64), B-half = [64,128).
# Groups 0-3 pair one A-tap with one B-tap (K=128); group 4 pairs the last
# A-tap with two x-taps; groups 5/6 are x-only (K=128 / K=96).
A_TAPS = [(0, 0), (0, 2), (1, 1), (2, 0), (2, 2)]   # -> G0..G4 @ p0-63
B_TAPS = [(0, 1), (1, 0), (1, 2), (2, 1)]           # -> G0..G3 @ p64-127
X_TAPS = [(0, 0), (0, 1),                            # -> G4 @ p64-95, p96-127
          (0, 2), (1, 0), (1, 1), (1, 2),            # -> G5 @ p0-127 (32 each)
          (2, 0), (2, 1), (2, 2)]                    # -> G6 @ p0-95


def _build(steps=T):
    nc = bacc.Bacc("TRN2", target_bir_lowering=False, debug=False)
    xp_d = nc.dram_tensor("xin", [BL, T, 9 * CIN, H, W], BF16, kind="ExternalInput")
    w_d = nc.dram_tensor("win", [128, NG * 2 * 128], BF16, kind="ExternalInput")
    b_d = nc.dram_tensor("bin", [128, 2], F32, kind="ExternalInput")
    o_d = nc.dram_tensor("out", [BL, HID, H, W], F32, kind="ExternalOutput")

    with tile.TileContext(nc) as tc:
        with ExitStack() as ctx:
            const = ctx.enter_context(tc.tile_pool(name="const", bufs=1))
            psum = ctx.enter_context(tc.tile_pool(name="psum", bufs=4, space="PSUM"))
            gp = ctx.enter_context(tc.tile_pool(name="gates", bufs=8))

            wsb = const.tile([128, NG * 2 * 128], BF16, tag="wsb")
            nc.sync.dma_start(out=wsb[:, :], in_=w_d[:, :])
            bsb = const.tile([128, 2], F32, tag="bsb")
            nc.sync.dma_start(out=bsb[:, :], in_=b_d[:, :])

            # Packed moving-operand tiles, one per (pass, image).
            gts = [
                [
                    const.tile([128, H, W], BF16, tag=f"gt{g}_{img}", name=f"gt{g}_{img}")
                    for img in range(BL)
                ]
                for g in range(NG)
            ]
            # Padded h plane, kept identically in both partition halves:
            # [64,128) is written directly by the h = o*tanh(c) mul; [0,64)
            # is bridged with one DMA per (t, img).
            hpad = [
                const.tile([128, PH, PW], BF16, tag=f"hpad{img}", name=f"hpad{img}")
                for img in range(BL)
            ]
            # Cell state lives in partitions [64,128) (lane-aligned with f/o).
            cst = const.tile([128, BL, H, W], F32, tag="cst")
            # cst needs no memset: t=0's pass 2 initializes it with a copy.
            for img in range(BL):
                nc.gpsimd.memset(hpad[img][:, :, :], 0.0)

            def stage_x(t, img):
                """Load the pre-shifted x-tap windows for timestep t."""
                nc.sync.dma_start(out=gts[4][img][64:128, :, :],
                                  in_=xp_d[img, t, 0:64, :, :])
                nc.sync.dma_start(out=gts[5][img][0:128, :, :],
                                  in_=xp_d[img, t, 64:192, :, :])
                nc.sync.dma_start(out=gts[6][img][0:96, :, :],
                                  in_=xp_d[img, t, 192:288, :, :])

            def stage(t, img):
                """Build the h-tap windows for timestep t from h_{t-1}."""
                # Bridge the new h plane to the A-half first so the A-window
                # copies can start as soon as possible.
                nc.sync.dma_start(out=hpad[img][0:64, :, :],
                                  in_=hpad[img][64:128, :, :])
                for g, (ky, kx) in enumerate(B_TAPS):
                    nc.vector.tensor_copy(
                        gts[g][img][64:128, :, :],
                        hpad[img][64:128, ky : ky + H, kx : kx + W],
                    )
                for g, (ky, kx) in enumerate(A_TAPS):
                    nc.vector.tensor_copy(
                        gts[g][img][0:64, :, :],
                        hpad[img][0:64, ky : ky + H, kx : kx + W],
                    )
                stage_x(t, img)

            for img in range(BL):
                stage_x(0, img)

            for t in range(steps):
                for img in range(BL):
                    # Pass 1 over row-groups: matmuls, PSUM-draining
                    # activations, the i*g product, its cross-half DMA
                    # bridge, and the in-place c *= f. Batching the bridges
                    # lets their ~2.5us DMA latencies overlap instead of
                    # stalling the in-order DVE queue once per row-group.
                    p1s, tg_sos = [], []
                    for rg in range(NRG):
                        y0 = rg * RG_ROWS
                        ps = [
                            psum.tile([128, RG_ROWS, 64], F32, tag=f"ps{c}", name=f"ps{c}")
                            for c in range(2)
                        ]
                        # At t=0, h == 0: skip the pure-h passes G0-G3 and
                        # read only G4's x-half, so no h windows are needed.
                        g_lo = 4 if t == 0 else 0
                        for g in range(g_lo, NG):
                            p0 = 64 if (t == 0 and g == 4) else 0
                            p1_ = 96 if g == NG - 1 else 128
                            for c in range(2):
                                nc.tensor.matmul(
                                    out=ps[c][:, :, :],
                                    lhsT=wsb[p0:p1_, (g * 2 + c) * 128 : (g * 2 + c + 1) * 128],
                                    rhs=gts[g][img][p0:p1_, y0 : y0 + RG_ROWS, :],
                                    start=(g == g_lo),
                                    stop=(g == NG - 1),
                                )

                        csl = cst[64:128, img, y0 : y0 + RG_ROWS, :]

                        sig_if = gp.tile([128, RG_ROWS, 64], BF16, tag="sig_if", bufs=4)
                        nc.scalar.activation(
                            out=sig_if[:, :, :], in_=ps[0][:, :, :],
                            func=AF.Sigmoid, bias=bsb[:, 0:1],
                        )
                        tg_so = gp.tile([128, RG_ROWS, 64], BF16, tag="tg_so")
                        nc.scalar.activation(
                            out=tg_so[0:64], in_=ps[1][0:64],
                            func=AF.Tanh, bias=bsb[0:64, 1:2],
                        )
                        nc.scalar.activation(
                            out=tg_so[64:128], in_=ps[1][64:128],
                            func=AF.Sigmoid, bias=bsb[64:128, 1:2],
                        )

                        p1 = gp.tile([128, RG_ROWS, 64], BF16, tag="p1")
                        nc.vector.tensor_mul(p1[0:64], sig_if[0:64], tg_so[0:64])
                        # Bridge i*g from partitions [0,64) to [64,128).
                        nc.sync.dma_start(out=p1[64:128], in_=p1[0:64])
                        if t > 0:
                            # c *= f in place (no tmp tile needed). At t=0,
                            # c == 0 and pass 2 initializes it with a copy.
                            nc.gpsimd.tensor_mul(csl, sig_if[64:128], csl)
                        p1s.append(p1)
                        tg_sos.append(tg_so)

                    # Pass 2: c += i*g, tanh, h (or the final leaky output).
                    for rg in range(NRG):
                        y0 = rg * RG_ROWS
                        csl = cst[64:128, img, y0 : y0 + RG_ROWS, :]
                        if t == 0:
                            nc.vector.tensor_copy(csl, p1s[rg][64:128])
                        else:
                            nc.vector.tensor_add(csl, csl, p1s[rg][64:128])
                        tct = gp.tile([128, RG_ROWS, 64], BF16, tag="tct", bufs=4)
                        nc.scalar.activation(tct[64:128], csl, func=AF.Tanh)

                        if t < steps - 1:
                            # h = sigmoid(o)*tanh(c), written straight into the
                            # padded plane's B-half (same partitions as o).
                            nc.vector.tensor_mul(
                                hpad[img][64:128, 1 + y0 : 1 + y0 + RG_ROWS, 1 : 1 + W],
                                tg_sos[rg][64:128], tct[64:128],
                            )
                        else:
                            ht = gp.tile([128, RG_ROWS, 64], F32, tag="ht", bufs=2)
                            nc.vector.tensor_mul(
                                ht[64:128], tg_sos[rg][64:128], tct[64:128]
                            )
                            ost = gp.tile([128, RG_ROWS, 64], F32, tag="ost", bufs=2)
                            nc.vector.scalar_tensor_tensor(
                                out=ost[64:128], in0=ht[64:128], scalar=0.01,
                                in1=ht[64:128], op0=ALU.mult, op1=ALU.max,
                            )
                            nc.sync.dma_start(
                                out=o_d[img, :, y0 : y0 + RG_ROWS, :],
                                in_=ost[64:128, :, :],
                            )
                    if t < steps - 1:
                        stage(t + 1, img)
    nc.compile()
    return nc


def _prep_weights(Wf, bf):
    # Gate order [i, f, o, g] -> [i, f, g, o]: chunkA=[i;f], chunkB=[g;o].
    perm = np.concatenate(
        [np.arange(128), np.arange(192, 256), np.arange(128, 192)]
    )
    Wp = np.asarray(Wf, np.float32)[perm]        # [256, CH, 3, 3]
    bp = np.asarray(bf, np.float32)[perm]
    # wl[p, g, c, m] = Wp[c*128+m, ch(p), ky(p), kx(p)] per the group layout.
    wl = np.zeros((128, NG, 2, 128), np.float32)
    for c in range(2):
        blk = Wp[c * 128 : (c + 1) * 128]        # [128, CH, 3, 3]
        for g, (ky, kx) in enumerate(A_TAPS):
            wl[0:64, g, c, :] = blk[:, CIN:CH, ky, kx].T
        for g, (ky, kx) in enumerate(B_TAPS):
            wl[64:128, g, c, :] = blk[:, CIN:CH, ky, kx].T
        for j, (ky, kx) in enumerate(X_TAPS):
            g = 4 + (j + 2) // 4                 # 0,1 -> G4; 2-5 -> G5; 6-8 -> G6
            p0 = (64 + 32 * j) if j < 2 else 32 * ((j - 2) % 4)
            wl[p0 : p0 + CIN, g, c, :] = blk[:, 0:CIN, ky, kx].T
    wl = np.ascontiguousarray(
        wl.reshape(128, NG * 2 * 128).astype(ml_dtypes.bfloat16)
    )
    b2 = np.ascontiguousarray(bp.reshape(2, 128).T)  # [128, 2]
    return wl, b2


def _prep_x(x):
    """Pre-shifted x-tap windows: [B, T, 9*CIN, H, W] bf16, X_TAPS order."""
    x = np.asarray(x, np.float32)
    xpad = np.zeros((B, T, CIN, PH, PW), np.float32)
    xpad[:, :, :, 1 : 1 + H, 1 : 1 + W] = x
    wins = [xpad[:, :, :, ky : ky + H, kx : kx + W] for (ky, kx) in X_TAPS]
    xp = np.concatenate(wins, axis=2)
    return np.ascontiguousarray(xp.astype(ml_dtypes.bfloat16))


_NC_CACHE = {}


def _get_nc():
    if "nc" not in _NC_CACHE:
        _NC_CACHE["nc"] = _build()
    return _NC_CACHE["nc"]


def _in_maps(x, Wf, bf):
    xp = _prep_x(x)
    wl, b2 = _prep_weights(Wf, bf)
    return [
        {
            "xin": np.ascontiguousarray(xp[i * BL : (i + 1) * BL]),
            "win": wl,
            "bin": b2,
        }
        for i in range(NCORES)
    ]


def _run(x, W, b, trace=False, **spmd_kwargs):
    nc = _get_nc()
    res = run_bass_kernel_spmd(
        nc, _in_maps(x, W, b), core_ids=list(range(NCORES)), trace=trace,
        **spmd_kwargs,
    )
    out = np.concatenate([res.results[i]["out"] for i in range(NCORES)], axis=0)
    return np.ascontiguousarray(out, dtype=np.float32), res


def kernel(x, W, b):
    out, _ = _run(x, W, b)
    return out
